# revision 39
# baseline (speedup 1.0000x reference)
"""Trainium2 Bass kernel for attention-pooling (nn_Attention_56607668961558).

Reference computation (per sample b):
    gbf_pj = gbf @ W_proj                      # [B, C]
    c[b,n] = (lcf[b,n,:] + gbf_pj[b,:]) @ W_pc # [B, N]
    a      = softmax(c, axis=n)
    ga[b,:] = sum_n a[b,n] * lcf[b,n,:]

Key identity: the gbf term contributes a constant per sample along n, and
softmax is shift-invariant, so `a` (and hence `ga`) do not depend on
gbf/W_proj at all.  The device kernel therefore only consumes lcf and W_pc.

Sharding: data-parallel over batch, 8 samples per core, W_pc replicated.

Per-core device plan (per sample, pipelined over 8 samples):
  - one 2 MB DMA loads the sample as [128p, (8j x 512c)] with n = p*8+j
    (16 KB contiguous HBM per partition -> near line-rate)
  - scores: per 512-wide chunk, one DVE scalar_tensor_tensor
    (out = lcf * W_pc, fused accum_out = sum over c)
    (tensor_tensor_reduce would be equivalent but crashes this runtime)
  - ScalarE exp with fused accum_out -> unnormalized exp(scores) + row sums
  - TensorE ones-matmul reduces row sums across partitions -> per-sample sum
  - pooling: 8 accumulating TensorE matmuls, exp(scores) column stationary,
    lcf tile moving -> unnormalized ga in PSUM; ScalarE copies to SBUF
  - softmax division happens on the host (exact same math, tiny arrays)
"""

import numpy as np

B, HH, WW, C = 64, 32, 32, 512
N = HH * WW            # 1024 spatial positions
NCORES = 8
BPC = B // NCORES      # 8 samples per core
NCH = N // 128         # 8 chunks of 128 spatial positions

_STATE = {}

LCF_BUFS = 6
PROD_BUFS = 3
PSUM_BUFS = 3
# Per-sample expS tiles + per-sample output DMAs instead of one persistent
# exps_all tile. Measured WORSE on HW (extra small DMAs cost more than the
# tile-hazard they remove) — keep False.
EXP_SPLIT = False
# In "hybrid" mode, offload pooling chunks to ACT/DVE for the first
# HYBRID_K samples of each pass; the rest run pure-PE. K balances the
# DVE wall (scores + offload adds) against the PE wall (chunk streams).
HYBRID_K = 8
# "hybridg": number of pooling chunks offloaded from PE (mults on ScalarE,
# running-sum adds chained on GPSIMD, one ones-matmul folds the result)
GA_OFF = 2


def _build(n_iter=1, pool_mode=None):
    """Build the per-core program. n_iter>1 repeats the whole computation
    (same inputs -> same outputs) so benchmarks can amortize dispatch
    overhead; results are identical to n_iter=1.

    pool_mode:
      "rhs"    — expS column stationary, lcf chunk streams as rhs (N=512)
      "stat"   — lcf [128n,128c] blocks stationary, expS column streams (N=1);
                 ga comes out c-on-partitions and is reassembled on the host
      "hybrid" — PE streams chunks 0..NCH-3; chunk NCH-2 is weighted on
                 ScalarE (per-partition scale), chunk NCH-1 is fused
                 multiply-add on DVE, and one ones-matmul folds the partial
                 into the same PSUM group (saves one fp32 PE stream/sample)
      "hybrid3g" — like hybrid but additionally: 2 score chunks move from
                 DVE to GPSIMD(mult)+ScalarE(accum), and a 3rd pooling
                 chunk moves off PE (ScalarE mult + GPSIMD add); spreads
                 the elementwise work across DVE/ACT/GPSIMD so every
                 engine sits near the DMA roofline
    """
    if pool_mode is None:
        pool_mode = POOL_MODE
    from contextlib import ExitStack

    from concourse import bacc, mybir
    import concourse.tile as tile

    f32 = mybir.dt.float32
    nc = bacc.Bacc(
        "TRN2", target_bir_lowering=False, debug=False, num_devices=NCORES
    )

    lcf = nc.dram_tensor("lcf", [BPC, 128, NCH * C], f32, kind="ExternalInput").ap()
    wpc = nc.dram_tensor("wpc", [128, C], f32, kind="ExternalInput").ap()
    ones = nc.dram_tensor("ones", [128, 1], f32, kind="ExternalInput").ap()
    exps_out = nc.dram_tensor(
        "exps_out", [128, BPC, NCH], f32, kind="ExternalOutput"
    ).ap()
    sums_out = nc.dram_tensor("sums_out", [1, BPC], f32, kind="ExternalOutput").ap()
    NCC = C // 128  # c-chunks for "stat" mode
    if pool_mode == "stat":
        ga_out = nc.dram_tensor(
            "ga_out", [128, BPC, NCC], f32, kind="ExternalOutput"
        ).ap()
    else:
        ga_out = nc.dram_tensor("ga_out", [1, BPC, C], f32, kind="ExternalOutput").ap()

    with tile.TileContext(nc) as tc, ExitStack() as ctx:
        const_pool = ctx.enter_context(tc.tile_pool(name="const", bufs=1))
        lcf_pool = ctx.enter_context(tc.tile_pool(name="lcfp", bufs=LCF_BUFS))
        prod_pool = ctx.enter_context(tc.tile_pool(name="prodp", bufs=PROD_BUFS))
        sc_pool = ctx.enter_context(tc.tile_pool(name="scp", bufs=2))
        outs_pool = ctx.enter_context(tc.tile_pool(name="outsp", bufs=1))
        ps_ga = ctx.enter_context(
            tc.tile_pool(name="psga", bufs=PSUM_BUFS, space="PSUM")
        )
        ps_sum = ctx.enter_context(tc.tile_pool(name="pssum", bufs=2, space="PSUM"))

        wpc_t = const_pool.tile([128, C], f32)
        nc.sync.dma_start(wpc_t[:], wpc[:])
        ones_t = const_pool.tile([128, 1], f32)
        nc.sync.dma_start(ones_t[:], ones[:])

        # touch Exp once up front so the ~2.7us ACT table load overlaps the
        # first lcf DMA instead of sitting on the first sample's critical path
        warm = const_pool.tile([1, 1], f32)
        nc.vector.memset(warm[:], 0.0)
        nc.scalar.activation(warm[:], warm[:], mybir.ActivationFunctionType.Exp)

        exps_all = None if EXP_SPLIT else outs_pool.tile([128, BPC, NCH], f32)
        sums_all = outs_pool.tile([1, BPC], f32)
        if pool_mode == "stat":
            ga_all = outs_pool.tile([128, BPC, NCC], f32)
        else:
            ga_all = outs_pool.tile([1, BPC, C], f32)

        for it in range(n_iter):
          for s in range(BPC):
            lt = lcf_pool.tile([128, NCH * C], f32, tag="lcf")
            nc.sync.dma_start(lt[:], lcf[s])

            scores = sc_pool.tile([128, NCH], f32, tag="scores")
            n_gp = 2 if pool_mode == "hybrid3g" else 0
            for j in range(NCH - n_gp):
                prod = prod_pool.tile([128, C], f32, tag="prod")
                nc.vector.scalar_tensor_tensor(
                    out=prod[:],
                    in0=lt[:, j * C : (j + 1) * C],
                    scalar=1.0,
                    in1=wpc_t[:],
                    op0=mybir.AluOpType.mult,
                    op1=mybir.AluOpType.mult,
                    accum_out=scores[:, j : j + 1],
                )
            for j in range(NCH - n_gp, NCH):
                prodg = prod_pool.tile([128, C], f32, tag="prodg")
                nc.gpsimd.tensor_tensor(
                    prodg[:],
                    lt[:, j * C : (j + 1) * C],
                    wpc_t[:],
                    op=mybir.AluOpType.mult,
                )
                nc.scalar.activation(
                    prodg[:],
                    prodg[:],
                    mybir.ActivationFunctionType.Copy,
                    accum_out=scores[:, j : j + 1],
                )

            expsum = sc_pool.tile([128, 1], f32, tag="expsum")
            if EXP_SPLIT:
                etile = sc_pool.tile([128, NCH], f32, tag="expS", bufs=3)
            else:
                etile = exps_all[:, s, :]
            nc.scalar.activation(
                etile[:],
                scores[:],
                mybir.ActivationFunctionType.Exp,
                accum_out=expsum[:],
            )
            if EXP_SPLIT:
                nc.sync.dma_start(exps_out[:, s, :], etile[:])

            sum_ps = ps_sum.tile([1, 1], f32)
            nc.tensor.matmul(sum_ps[:], ones_t[:], expsum[:], start=True, stop=True)
            nc.scalar.copy(sums_all[:, s : s + 1], sum_ps[:])

            if pool_mode == "rhs" or (pool_mode == "hybrid" and s >= HYBRID_K):
                ga_ps = ps_ga.tile([1, C], f32)
                for j in range(NCH):
                    nc.tensor.matmul(
                        ga_ps[:],
                        etile[:, j : j + 1],
                        lt[:, j * C : (j + 1) * C],
                        start=(j == 0),
                        stop=(j == NCH - 1),
                    )
                nc.scalar.copy(ga_all[:, s, :], ga_ps[:])
            elif pool_mode in ("hybrid", "hybrid3g"):
                n_off = 2 if pool_mode == "hybrid" else 3
                ga_ps = ps_ga.tile([1, C], f32)
                for j in range(NCH - n_off):
                    nc.tensor.matmul(
                        ga_ps[:],
                        etile[:, j : j + 1],
                        lt[:, j * C : (j + 1) * C],
                        start=(j == 0),
                        stop=False,
                    )
                # chunk A: weighted on ScalarE (per-partition scale)
                ja = NCH - n_off
                partial = prod_pool.tile([128, C], f32, tag="partial")
                nc.scalar.activation(
                    partial[:],
                    lt[:, ja * C : (ja + 1) * C],
                    mybir.ActivationFunctionType.Copy,
                    scale=etile[:, ja : ja + 1],
                )
                # chunk B: fused multiply-add on DVE
                jb = ja + 1
                partial2 = prod_pool.tile([128, C], f32, tag="partial2")
                nc.vector.scalar_tensor_tensor(
                    out=partial2[:],
                    in0=lt[:, jb * C : (jb + 1) * C],
                    scalar=etile[:, jb : jb + 1],
                    in1=partial[:],
                    op0=mybir.AluOpType.mult,
                    op1=mybir.AluOpType.add,
                )
                last = partial2
                if pool_mode == "hybrid3g":
                    # chunk C: weighted on ScalarE, accumulated on GPSIMD
                    jc = jb + 1
                    prodc = prod_pool.tile([128, C], f32, tag="prodc")
                    nc.scalar.activation(
                        prodc[:],
                        lt[:, jc * C : (jc + 1) * C],
                        mybir.ActivationFunctionType.Copy,
                        scale=etile[:, jc : jc + 1],
                    )
                    partial3 = prod_pool.tile([128, C], f32, tag="partial3")
                    nc.gpsimd.tensor_tensor(
                        partial3[:], prodc[:], partial2[:], op=mybir.AluOpType.add
                    )
                    last = partial3
                nc.tensor.matmul(
                    ga_ps[:], ones_t[:], last[:], start=False, stop=True
                )
                nc.scalar.copy(ga_all[:, s, :], ga_ps[:])
            elif pool_mode == "hybridg":
                ga_ps = ps_ga.tile([1, C], f32)
                for j in range(NCH - GA_OFF):
                    nc.tensor.matmul(
                        ga_ps[:],
                        etile[:, j : j + 1],
                        lt[:, j * C : (j + 1) * C],
                        start=(j == 0),
                        stop=False,
                    )
                last = None
                for j in range(NCH - GA_OFF, NCH):
                    prodj = prod_pool.tile([128, C], f32, tag=f"gprod{j % 2}")
                    nc.scalar.activation(
                        prodj[:],
                        lt[:, j * C : (j + 1) * C],
                        mybir.ActivationFunctionType.Copy,
                        scale=etile[:, j : j + 1],
                    )
                    if last is None:
                        last = prodj
                    else:
                        acc = prod_pool.tile([128, C], f32, tag=f"gacc{j % 2}")
                        nc.gpsimd.tensor_tensor(
                            acc[:], last[:], prodj[:], op=mybir.AluOpType.add
                        )
                        last = acc
                nc.tensor.matmul(
                    ga_ps[:], ones_t[:], last[:], start=False, stop=True
                )
                nc.scalar.copy(ga_all[:, s, :], ga_ps[:])
            else:
                ga_ps = ps_ga.tile([128, NCC], f32)
                for k in range(NCC):
                    for j in range(NCH):
                        nc.tensor.matmul(
                            ga_ps[:, k : k + 1],
                            lt[:, j * C + k * 128 : j * C + (k + 1) * 128],
                            etile[:, j : j + 1],
                            start=(j == 0),
                            stop=(j == NCH - 1),
                        )
                nc.scalar.copy(ga_all[:, s, :], ga_ps[:])

        if not EXP_SPLIT:
            nc.sync.dma_start(exps_out[:], exps_all[:])
        nc.sync.dma_start(sums_out[:], sums_all[:])
        nc.sync.dma_start(ga_out[:], ga_all[:])

    nc.compile()
    return nc


POOL_MODE = "hybridg"


def kernel(lcf, gbf, W_proj, W_pc):
    from concourse.bass_utils import run_bass_kernel_spmd

    if _STATE.get("pool_mode") != POOL_MODE:
        _STATE["nc"] = _build(pool_mode=POOL_MODE)
        _STATE["pool_mode"] = POOL_MODE
    nc = _STATE["nc"]

    lcf = np.ascontiguousarray(lcf, dtype=np.float32).reshape(B, N, C)
    wpc_rep = np.ascontiguousarray(
        np.broadcast_to(
            np.asarray(W_pc, dtype=np.float32).reshape(1, C), (128, C)
        )
    )
    ones = np.ones((128, 1), dtype=np.float32)

    in_maps = []
    for i in range(NCORES):
        # [BPC, N, C] -> [BPC, 128p, NCHj, C] with n = p*NCH + j -> flatten (j, c)
        shard = lcf[i * BPC : (i + 1) * BPC].reshape(BPC, 128, NCH, C)
        shard = np.ascontiguousarray(shard).reshape(BPC, 128, NCH * C)
        in_maps.append({"lcf": shard, "wpc": wpc_rep, "ones": ones})

    res = run_bass_kernel_spmd(nc, in_maps, list(range(NCORES)))
    _STATE["last_results"] = res

    ga = np.empty((B, C), dtype=np.float32)
    a = np.empty((B, N), dtype=np.float32)
    for i in range(NCORES):
        r = res.results[i]
        exps = np.asarray(r["exps_out"]).reshape(128, BPC, NCH)  # [p, s, j]
        sums = np.asarray(r["sums_out"]).reshape(BPC)            # [s]
        if POOL_MODE == "rhs":
            gar = np.asarray(r["ga_out"]).reshape(BPC, C)        # [s, c]
        else:
            # [p, s, k] with c = k*128 + p -> [s, k, p] -> [s, c]
            gar = np.asarray(r["ga_out"]).transpose(1, 2, 0).reshape(BPC, C)
        # a[s, n] with n = p*NCH + j
        a_un = exps.transpose(1, 0, 2).reshape(BPC, N)
        a[i * BPC : (i + 1) * BPC] = a_un / sums[:, None]
        ga[i * BPC : (i + 1) * BPC] = gar / sums[:, None]

    return ga, a.reshape(B, HH, WW)


# revision 43
# speedup vs baseline: 1.1439x; 1.1439x over previous
"""Trainium2 Bass kernel for attention-pooling (nn_Attention_56607668961558).

Reference computation (per sample b):
    gbf_pj = gbf @ W_proj                      # [B, C]
    c[b,n] = (lcf[b,n,:] + gbf_pj[b,:]) @ W_pc # [B, N]
    a      = softmax(c, axis=n)
    ga[b,:] = sum_n a[b,n] * lcf[b,n,:]

Key identity: the gbf term contributes a constant per sample along n, and
softmax is shift-invariant, so `a` (and hence `ga`) do not depend on
gbf/W_proj at all.  The device kernel therefore only consumes lcf and W_pc.

Sharding: data-parallel over batch, 8 samples per core, W_pc replicated.

Per-core device plan (per sample, pipelined over 8 samples):
  - one 2 MB DMA loads the sample as [128p, (8j x 512c)] with n = p*8+j
    (16 KB contiguous HBM per partition -> near line-rate)
  - scores: per 512-wide chunk, one DVE scalar_tensor_tensor
    (out = lcf * W_pc, fused accum_out = sum over c)
    (tensor_tensor_reduce would be equivalent but crashes this runtime)
  - ScalarE exp with fused accum_out -> unnormalized exp(scores) + row sums
  - TensorE ones-matmul reduces row sums across partitions -> per-sample sum
  - pooling ("hybridg"): 6 accumulating TensorE matmuls (exp column
    stationary, lcf streaming at fp32's 4 cyc/col); the last 2 chunks are
    weighted on ScalarE (per-partition scale), summed on GPSIMD, and folded
    into the same PSUM group by one ones-matmul — balancing PE against the
    DVE scores pass; ScalarE evicts ga PSUM->SBUF
  - softmax division happens on the host (exact same math, tiny arrays)

Measured ~47 us/pass per core on HW (paired-delta method) vs a ~42 us
DMA-only floor for the same 16.8 MB stream; exact-fp32 throughout
(rel err ~5e-6 vs the jax reference).
"""

import numpy as np

B, HH, WW, C = 64, 32, 32, 512
N = HH * WW            # 1024 spatial positions
NCORES = 8
BPC = B // NCORES      # 8 samples per core
NCH = N // 128         # 8 chunks of 128 spatial positions

_STATE = {}

LCF_BUFS = 6
PROD_BUFS = 3
PSUM_BUFS = 3
# Per-sample expS tiles + per-sample output DMAs instead of one persistent
# exps_all tile. Measured WORSE on HW (extra small DMAs cost more than the
# tile-hazard they remove) — keep False.
EXP_SPLIT = False
# In "hybrid" mode, offload pooling chunks to ACT/DVE for the first
# HYBRID_K samples of each pass; the rest run pure-PE. K balances the
# DVE wall (scores + offload adds) against the PE wall (chunk streams).
HYBRID_K = 8
# "hybridg": number of pooling chunks offloaded from PE (mults on ScalarE,
# running-sum adds chained on GPSIMD, one ones-matmul folds the result)
GA_OFF = 2
# Move one score chunk per sample to GPSIMD(mult)+ScalarE(accum) to unload DVE
SC_GP = 0
# For the first DVE3_K samples, offload a 3rd pooling chunk via a DVE
# fused multiply-add (rebalances PE against the lightened DVE)
DVE3_K = 0


def _build(n_iter=1, pool_mode=None):
    """Build the per-core program. n_iter>1 repeats the whole computation
    (same inputs -> same outputs) so benchmarks can amortize dispatch
    overhead; results are identical to n_iter=1.

    pool_mode:
      "rhs"    — expS column stationary, lcf chunk streams as rhs (N=512)
      "stat"   — lcf [128n,128c] blocks stationary, expS column streams (N=1);
                 ga comes out c-on-partitions and is reassembled on the host
      "hybrid" — PE streams chunks 0..NCH-3; chunk NCH-2 is weighted on
                 ScalarE (per-partition scale), chunk NCH-1 is fused
                 multiply-add on DVE, and one ones-matmul folds the partial
                 into the same PSUM group (saves one fp32 PE stream/sample)
      "hybrid3g" — like hybrid but additionally: 2 score chunks move from
                 DVE to GPSIMD(mult)+ScalarE(accum), and a 3rd pooling
                 chunk moves off PE (ScalarE mult + GPSIMD add); spreads
                 the elementwise work across DVE/ACT/GPSIMD so every
                 engine sits near the DMA roofline
    """
    if pool_mode is None:
        pool_mode = POOL_MODE
    from contextlib import ExitStack

    from concourse import bacc, mybir
    import concourse.tile as tile

    f32 = mybir.dt.float32
    nc = bacc.Bacc(
        "TRN2", target_bir_lowering=False, debug=False, num_devices=NCORES
    )

    lcf = nc.dram_tensor("lcf", [BPC, 128, NCH * C], f32, kind="ExternalInput").ap()
    wpc = nc.dram_tensor("wpc", [128, C], f32, kind="ExternalInput").ap()
    ones = nc.dram_tensor("ones", [128, 1], f32, kind="ExternalInput").ap()
    exps_out = nc.dram_tensor(
        "exps_out", [128, BPC, NCH], f32, kind="ExternalOutput"
    ).ap()
    sums_out = nc.dram_tensor("sums_out", [1, BPC], f32, kind="ExternalOutput").ap()
    NCC = C // 128  # c-chunks for "stat" mode
    if pool_mode == "stat":
        ga_out = nc.dram_tensor(
            "ga_out", [128, BPC, NCC], f32, kind="ExternalOutput"
        ).ap()
    else:
        ga_out = nc.dram_tensor("ga_out", [1, BPC, C], f32, kind="ExternalOutput").ap()

    with tile.TileContext(nc) as tc, ExitStack() as ctx:
        const_pool = ctx.enter_context(tc.tile_pool(name="const", bufs=1))
        lcf_pool = ctx.enter_context(tc.tile_pool(name="lcfp", bufs=LCF_BUFS))
        prod_pool = ctx.enter_context(tc.tile_pool(name="prodp", bufs=PROD_BUFS))
        sc_pool = ctx.enter_context(tc.tile_pool(name="scp", bufs=2))
        outs_pool = ctx.enter_context(tc.tile_pool(name="outsp", bufs=1))
        ps_ga = ctx.enter_context(
            tc.tile_pool(name="psga", bufs=PSUM_BUFS, space="PSUM")
        )
        ps_sum = ctx.enter_context(tc.tile_pool(name="pssum", bufs=2, space="PSUM"))

        wpc_t = const_pool.tile([128, C], f32)
        nc.sync.dma_start(wpc_t[:], wpc[:])
        ones_t = const_pool.tile([128, 1], f32)
        nc.sync.dma_start(ones_t[:], ones[:])

        # touch Exp once up front so the ~2.7us ACT table load overlaps the
        # first lcf DMA instead of sitting on the first sample's critical path
        warm = const_pool.tile([1, 1], f32)
        nc.vector.memset(warm[:], 0.0)
        nc.scalar.activation(warm[:], warm[:], mybir.ActivationFunctionType.Exp)

        exps_all = None if EXP_SPLIT else outs_pool.tile([128, BPC, NCH], f32)
        sums_all = outs_pool.tile([1, BPC], f32)
        if pool_mode == "stat":
            ga_all = outs_pool.tile([128, BPC, NCC], f32)
        else:
            ga_all = outs_pool.tile([1, BPC, C], f32)

        for it in range(n_iter):
          for s in range(BPC):
            lt = lcf_pool.tile([128, NCH * C], f32, tag="lcf")
            nc.sync.dma_start(lt[:], lcf[s])

            scores = sc_pool.tile([128, NCH], f32, tag="scores")
            if pool_mode == "hybridg" and SC_GP:
                # chunk 0's scores via GPSIMD mult + ScalarE fused accum
                prodg = prod_pool.tile([128, C], f32, tag="prodg")
                nc.gpsimd.tensor_tensor(
                    prodg[:], lt[:, 0:C], wpc_t[:], op=mybir.AluOpType.mult
                )
                nc.scalar.activation(
                    prodg[:],
                    prodg[:],
                    mybir.ActivationFunctionType.Copy,
                    accum_out=scores[:, 0:1],
                )
            sc_start = 1 if (pool_mode == "hybridg" and SC_GP) else 0
            n_gp = 2 if pool_mode == "hybrid3g" else 0
            for j in range(sc_start, NCH - n_gp):
                prod = prod_pool.tile([128, C], f32, tag="prod")
                nc.vector.scalar_tensor_tensor(
                    out=prod[:],
                    in0=lt[:, j * C : (j + 1) * C],
                    scalar=1.0,
                    in1=wpc_t[:],
                    op0=mybir.AluOpType.mult,
                    op1=mybir.AluOpType.mult,
                    accum_out=scores[:, j : j + 1],
                )
            for j in range(NCH - n_gp, NCH):
                prodg = prod_pool.tile([128, C], f32, tag="prodg")
                nc.gpsimd.tensor_tensor(
                    prodg[:],
                    lt[:, j * C : (j + 1) * C],
                    wpc_t[:],
                    op=mybir.AluOpType.mult,
                )
                nc.scalar.activation(
                    prodg[:],
                    prodg[:],
                    mybir.ActivationFunctionType.Copy,
                    accum_out=scores[:, j : j + 1],
                )

            expsum = sc_pool.tile([128, 1], f32, tag="expsum")
            if EXP_SPLIT:
                etile = sc_pool.tile([128, NCH], f32, tag="expS", bufs=3)
            else:
                etile = exps_all[:, s, :]
            nc.scalar.activation(
                etile[:],
                scores[:],
                mybir.ActivationFunctionType.Exp,
                accum_out=expsum[:],
            )
            if EXP_SPLIT:
                nc.sync.dma_start(exps_out[:, s, :], etile[:])

            sum_ps = ps_sum.tile([1, 1], f32)
            nc.tensor.matmul(sum_ps[:], ones_t[:], expsum[:], start=True, stop=True)
            nc.scalar.copy(sums_all[:, s : s + 1], sum_ps[:])

            if pool_mode == "rhs" or (pool_mode == "hybrid" and s >= HYBRID_K):
                ga_ps = ps_ga.tile([1, C], f32)
                for j in range(NCH):
                    nc.tensor.matmul(
                        ga_ps[:],
                        etile[:, j : j + 1],
                        lt[:, j * C : (j + 1) * C],
                        start=(j == 0),
                        stop=(j == NCH - 1),
                    )
                nc.scalar.copy(ga_all[:, s, :], ga_ps[:])
            elif pool_mode in ("hybrid", "hybrid3g"):
                n_off = 2 if pool_mode == "hybrid" else 3
                ga_ps = ps_ga.tile([1, C], f32)
                for j in range(NCH - n_off):
                    nc.tensor.matmul(
                        ga_ps[:],
                        etile[:, j : j + 1],
                        lt[:, j * C : (j + 1) * C],
                        start=(j == 0),
                        stop=False,
                    )
                # chunk A: weighted on ScalarE (per-partition scale)
                ja = NCH - n_off
                partial = prod_pool.tile([128, C], f32, tag="partial")
                nc.scalar.activation(
                    partial[:],
                    lt[:, ja * C : (ja + 1) * C],
                    mybir.ActivationFunctionType.Copy,
                    scale=etile[:, ja : ja + 1],
                )
                # chunk B: fused multiply-add on DVE
                jb = ja + 1
                partial2 = prod_pool.tile([128, C], f32, tag="partial2")
                nc.vector.scalar_tensor_tensor(
                    out=partial2[:],
                    in0=lt[:, jb * C : (jb + 1) * C],
                    scalar=etile[:, jb : jb + 1],
                    in1=partial[:],
                    op0=mybir.AluOpType.mult,
                    op1=mybir.AluOpType.add,
                )
                last = partial2
                if pool_mode == "hybrid3g":
                    # chunk C: weighted on ScalarE, accumulated on GPSIMD
                    jc = jb + 1
                    prodc = prod_pool.tile([128, C], f32, tag="prodc")
                    nc.scalar.activation(
                        prodc[:],
                        lt[:, jc * C : (jc + 1) * C],
                        mybir.ActivationFunctionType.Copy,
                        scale=etile[:, jc : jc + 1],
                    )
                    partial3 = prod_pool.tile([128, C], f32, tag="partial3")
                    nc.gpsimd.tensor_tensor(
                        partial3[:], prodc[:], partial2[:], op=mybir.AluOpType.add
                    )
                    last = partial3
                nc.tensor.matmul(
                    ga_ps[:], ones_t[:], last[:], start=False, stop=True
                )
                nc.scalar.copy(ga_all[:, s, :], ga_ps[:])
            elif pool_mode == "hybridg":
                n_off = GA_OFF + (1 if s < DVE3_K else 0)
                ga_ps = ps_ga.tile([1, C], f32)
                for j in range(NCH - n_off):
                    nc.tensor.matmul(
                        ga_ps[:],
                        etile[:, j : j + 1],
                        lt[:, j * C : (j + 1) * C],
                        start=(j == 0),
                        stop=False,
                    )
                # the first GA_OFF offloaded chunks: ScalarE mults, one
                # GPSIMD add combines them
                last = None
                for j in range(NCH - n_off, NCH - n_off + GA_OFF):
                    prodj = prod_pool.tile([128, C], f32, tag=f"gprod{j % 2}")
                    nc.scalar.activation(
                        prodj[:],
                        lt[:, j * C : (j + 1) * C],
                        mybir.ActivationFunctionType.Copy,
                        scale=etile[:, j : j + 1],
                    )
                    if last is None:
                        last = prodj
                    else:
                        acc = prod_pool.tile([128, C], f32, tag=f"gacc{j % 2}")
                        nc.gpsimd.tensor_tensor(
                            acc[:], last[:], prodj[:], op=mybir.AluOpType.add
                        )
                        last = acc
                # optional extra chunk: DVE fused multiply-add
                for j in range(NCH - n_off + GA_OFF, NCH):
                    acc2 = prod_pool.tile([128, C], f32, tag="dacc")
                    nc.vector.scalar_tensor_tensor(
                        out=acc2[:],
                        in0=lt[:, j * C : (j + 1) * C],
                        scalar=etile[:, j : j + 1],
                        in1=last[:],
                        op0=mybir.AluOpType.mult,
                        op1=mybir.AluOpType.add,
                    )
                    last = acc2
                nc.tensor.matmul(
                    ga_ps[:], ones_t[:], last[:], start=False, stop=True
                )
                nc.scalar.copy(ga_all[:, s, :], ga_ps[:])
            else:
                ga_ps = ps_ga.tile([128, NCC], f32)
                for k in range(NCC):
                    for j in range(NCH):
                        nc.tensor.matmul(
                            ga_ps[:, k : k + 1],
                            lt[:, j * C + k * 128 : j * C + (k + 1) * 128],
                            etile[:, j : j + 1],
                            start=(j == 0),
                            stop=(j == NCH - 1),
                        )
                nc.scalar.copy(ga_all[:, s, :], ga_ps[:])

        if not EXP_SPLIT:
            nc.sync.dma_start(exps_out[:], exps_all[:])
        nc.sync.dma_start(sums_out[:], sums_all[:])
        nc.sync.dma_start(ga_out[:], ga_all[:])

    nc.compile()
    return nc


POOL_MODE = "hybridg"


def kernel(lcf, gbf, W_proj, W_pc):
    from concourse.bass_utils import run_bass_kernel_spmd

    if _STATE.get("pool_mode") != POOL_MODE:
        _STATE["nc"] = _build(pool_mode=POOL_MODE)
        _STATE["pool_mode"] = POOL_MODE
    nc = _STATE["nc"]

    lcf = np.ascontiguousarray(lcf, dtype=np.float32).reshape(B, N, C)
    wpc_rep = np.ascontiguousarray(
        np.broadcast_to(
            np.asarray(W_pc, dtype=np.float32).reshape(1, C), (128, C)
        )
    )
    ones = np.ones((128, 1), dtype=np.float32)

    in_maps = []
    for i in range(NCORES):
        # [BPC, N, C] -> [BPC, 128p, NCHj, C] with n = p*NCH + j -> flatten (j, c)
        shard = lcf[i * BPC : (i + 1) * BPC].reshape(BPC, 128, NCH, C)
        shard = np.ascontiguousarray(shard).reshape(BPC, 128, NCH * C)
        in_maps.append({"lcf": shard, "wpc": wpc_rep, "ones": ones})

    res = run_bass_kernel_spmd(nc, in_maps, list(range(NCORES)))
    _STATE["last_results"] = res

    ga = np.empty((B, C), dtype=np.float32)
    a = np.empty((B, N), dtype=np.float32)
    for i in range(NCORES):
        r = res.results[i]
        exps = np.asarray(r["exps_out"]).reshape(128, BPC, NCH)  # [p, s, j]
        sums = np.asarray(r["sums_out"]).reshape(BPC)            # [s]
        if POOL_MODE == "rhs":
            gar = np.asarray(r["ga_out"]).reshape(BPC, C)        # [s, c]
        else:
            # [p, s, k] with c = k*128 + p -> [s, k, p] -> [s, c]
            gar = np.asarray(r["ga_out"]).transpose(1, 2, 0).reshape(BPC, C)
        # a[s, n] with n = p*NCH + j
        a_un = exps.transpose(1, 0, 2).reshape(BPC, N)
        a[i * BPC : (i + 1) * BPC] = a_un / sums[:, None]
        ga[i * BPC : (i + 1) * BPC] = gar / sums[:, None]

    return ga, a.reshape(B, HH, WW)
